# revision 8
# baseline (speedup 1.0000x reference)
"""Multi-head causal self-attention (B=4, T=2048, C=1024, H=16, Dh=64) on 8
Trainium2 NeuronCores.

Reference semantics (note the intentional inverse scale: scores are
MULTIPLIED by sqrt(Dh)=8):
    qkv = x @ W_qkv;  q,k,v = split(qkv)
    scores = (q @ k^T) * 8, causal-masked, softmax
    y = attn @ v;  out = y @ W_proj

Sharding: batch (4) x head-halves (2) -> 8 cores. Core c handles batch c//2
and heads [8*(c%2), 8*(c%2)+8). Each core computes its y slice [T, 512] and a
partial projection y_slice @ W_proj[rows] -> [T, 1024]; the host sums the two
partials per batch.

Precision: the q/k path (x, Wq, Wk, q@k^T) runs in float32r (fast fp32 mode on
the PE; ~TF32 accuracy), which the softmax's x8 logit amplification requires.
v / attn-weights / output projection run in bf16 (benign linear errors).
The x8 scale is folded into Wq on the host (exact: power of two).

Softmax: per 128-row block, scores accumulate in PSUM fp32; row max via DVE
reduce_max (negated -> exp bias); exp on ACT writes bf16 attn weights and
accumulates the fp32 row sum (denominator) in the same instruction; attn
blocks are transposed on the PE (via identity matmul) for the attn @ v
matmul; normalization by 1/denominator is applied to the [128, 64] y block.
"""

import os
from contextlib import ExitStack

import numpy as np

import orjson

import concourse.bass as bass
import concourse.bass2jax as bass2jax
import concourse.mybir as mybir
import concourse.tile as tile
from concourse import bass_utils
from concourse.masks import make_causal_mask, make_identity

F32 = mybir.dt.float32
F32R = mybir.dt.float32r
BF16 = mybir.dt.bfloat16
AF = mybir.ActivationFunctionType
AX = mybir.AxisListType
OP = mybir.AluOpType

P = 128
C_IN = 1024
DQ = 512  # per-core head dims (8 heads x 64)
DH = 64
N_HEADS_LOC = 8
MASK_VAL = -1e30

# ---------------------------------------------------------------------------
# Walrus workaround: this container's walrus accepts at most ONE sync wait per
# instruction, but Tile emits instructions with several (most notably the tail
# drain). Split them at the BIR-JSON level into same-engine NoOps carrying one
# wait each, inserted right before the offending instruction.
# ---------------------------------------------------------------------------

_orig_compile_bir_kernel = bass_utils.compile_bir_kernel
_wsplit_counter = [0]


def _split_multi_waits_json(bir: dict) -> None:
    for fn in bir.get("functions", []):
        for bb in fn.get("blocks", []):
            insts = bb.get("instructions", [])
            if not any(
                len((i.get("sync_info") or {}).get("on_wait") or []) > 1
                for i in insts
            ):
                continue
            out = []
            for ins in insts:
                si = ins.get("sync_info")
                ow = (si or {}).get("on_wait") or []
                if len(ow) > 1:
                    for w in ow[:-1]:
                        _wsplit_counter[0] += 1
                        nop = {
                            "engine": ins["engine"],
                            "ins": [],
                            "name": f"WSPLIT-{_wsplit_counter[0]}",
                            "opcode": "NoOp",
                            "outs": [],
                            "sync_info": {"on_update": [], "on_wait": [w]},
                        }
                        if "debug" in ins:
                            nop["debug"] = ins["debug"]
                        out.append(nop)
                    si["on_wait"] = [ow[-1]]
                out.append(ins)
            bb["instructions"] = out


def _patched_compile_bir_kernel(bir_json, tmpdir, neff_name="file.neff"):
    bir = orjson.loads(bir_json)
    _split_multi_waits_json(bir)
    return _orig_compile_bir_kernel(orjson.dumps(bir), tmpdir, neff_name=neff_name)


bass2jax.compile_bir_kernel = _patched_compile_bir_kernel

if os.environ.get("ATTN_LDW_OPT") == "1":
    _orig_run_command = bass_utils.run_command

    def _patched_run_command(argv, **kw):
        argv = [
            "--enable-ldw-opt=true" if a == "--enable-ldw-opt=false" else a
            for a in argv
        ]
        return _orig_run_command(argv, **kw)

    bass_utils.run_command = _patched_run_command


# ---------------------------------------------------------------------------
# Kernel builder
# ---------------------------------------------------------------------------


def build_attention_nc(T: int) -> bass.Bass:
    NT = T // P  # 128-row blocks along the sequence
    assert NT % 4 == 0 and NT % min(8, NT) == 0

    nc = bass.Bass("TRN2", target_bir_lowering=False, debug=False)
    x_d = nc.dram_tensor("x", [T, C_IN], F32, kind="ExternalInput").ap()
    wq_d = nc.dram_tensor("wq", [C_IN, DQ], F32, kind="ExternalInput").ap()
    wk_d = nc.dram_tensor("wk", [C_IN, DQ], F32, kind="ExternalInput").ap()
    wv_d = nc.dram_tensor("wv", [C_IN, DQ], F32, kind="ExternalInput").ap()
    wp_d = nc.dram_tensor("wp", [DQ, C_IN], F32, kind="ExternalInput").ap()
    out_d = nc.dram_tensor("out", [T, C_IN], F32, kind="ExternalOutput").ap()

    eng_ctr = [0]

    with tile.TileContext(nc) as tc, ExitStack() as ctx:
        def copy_psum(dst, src):
            # Alternate PSUM->SBUF copies between ACT and DVE to balance load.
            if eng_ctr[0] % 2 == 0:
                nc.scalar.copy(dst, src)
            else:
                nc.vector.tensor_copy(dst, src)
            eng_ctr[0] += 1

        const = ctx.enter_context(tc.tile_pool(name="const", bufs=1))
        ident_f = const.tile([P, P], F32, name="ident_f")
        ident_b = const.tile([P, P], BF16, name="ident_b")
        causal = const.tile([P, P], F32, name="causal")
        make_identity(nc, ident_f)
        make_identity(nc, ident_b)
        make_causal_mask(nc, causal, mask_val=MASK_VAL)

        persist = ctx.enter_context(tc.tile_pool(name="persist", bufs=1))
        # qkT[2j] = q^T rows for head pair j, qkT[2j+1] = k^T. Layout: [128, T]
        # f32r; head 2j on partitions 0..63, head 2j+1 on 64..127.
        qkT = [
            persist.tile([P, T], F32R, name=f"qkT{j}", tag=f"qkT{j}")
            for j in range(8)
        ]
        v_sb = [
            persist.tile([P, DQ], BF16, name=f"v{t}", tag=f"v{t}")
            for t in range(NT)
        ]

        # ------------------- Phase 1: x^T, q^T, k^T, v --------------------
        with tc.tile_pool(name="xstage", bufs=1) as xstage, \
             tc.tile_pool(name="xTp", bufs=1) as xTp, \
             tc.tile_pool(name="wst", bufs=4) as wst, \
             tc.tile_pool(name="wrp", bufs=16) as wrp, \
             tc.tile_pool(name="wvp", bufs=1) as wvp, \
             tc.tile_pool(name="ps_x", bufs=3, space="PSUM") as ps_x, \
             tc.tile_pool(name="ps_mm", bufs=5, space="PSUM") as ps_mm:
            xT = [
                xTp.tile([P, T], F32R, name=f"xT{c}", tag=f"xT{c}")
                for c in range(8)
            ]
            # x^T via PE transposes, batched 4 t-blocks per PSUM tile.
            for tg in range(NT // 4):
                xts = []
                for i in range(4):
                    t = tg * 4 + i
                    xt = xstage.tile([P, C_IN], F32, name="xt", tag=f"xt{i}")
                    nc.sync.dma_start(out=xt, in_=x_d[t * P:(t + 1) * P, :])
                    xts.append(xt)
                for c in range(8):
                    px = ps_x.tile([P, 512], F32, name="px", tag="px")
                    for i in range(4):
                        nc.tensor.transpose(
                            px[:, i * P:(i + 1) * P],
                            xts[i][:, c * P:(c + 1) * P],
                            ident_f,
                        )
                    # f32r destination: DVE rounds on copy.
                    nc.vector.tensor_copy(
                        xT[c][:, tg * 512:(tg + 1) * 512], px
                    )

            # q^T / k^T: W stationary (f32r), x^T moving. dblk j covers head
            # pair j; emit q then k per pair so early heads unblock first.
            for j in range(4):
                for wdram, dst_idx in ((wq_d, 2 * j), (wk_d, 2 * j + 1)):
                    dst = qkT[dst_idx]
                    pts = [
                        ps_mm.tile([P, 512], F32, name="pqk", tag="pqk")
                        for _ in range(T // 512)
                    ]
                    for c in range(8):
                        ws = wst.tile([P, P], F32, name="ws", tag="ws")
                        nc.sync.dma_start(
                            out=ws,
                            in_=wdram[c * P:(c + 1) * P, j * P:(j + 1) * P],
                        )
                        wr = wrp.tile([P, P], F32R, name="wr", tag="wr")
                        nc.vector.tensor_copy(wr, ws)
                        for tch in range(T // 512):
                            nc.tensor.matmul(
                                pts[tch],
                                wr,
                                xT[c][:, tch * 512:(tch + 1) * 512],
                                start=(c == 0),
                                stop=(c == 7),
                            )
                    for tch in range(T // 512):
                        nc.vector.tensor_copy(
                            dst[:, tch * 512:(tch + 1) * 512], pts[tch]
                        )

            # v (t-major): x^T stationary, Wv moving.
            wvr = []
            for c in range(8):
                wvs = wst.tile([P, DQ], F32, name="wvs", tag="wvs")
                nc.sync.dma_start(out=wvs, in_=wv_d[c * P:(c + 1) * P, :])
                wr_ = wvp.tile([P, DQ], F32R, name=f"wvr{c}", tag=f"wvr{c}")
                nc.vector.tensor_copy(wr_, wvs)
                wvr.append(wr_)
            for t in range(NT):
                pv = ps_mm.tile([P, DQ], F32, name="pv", tag="pqk")
                for c in range(8):
                    nc.tensor.matmul(
                        pv,
                        xT[c][:, t * P:(t + 1) * P],
                        wvr[c],
                        start=(c == 0),
                        stop=(c == 7),
                    )
                copy_psum(v_sb[t], pv)

        # yT opened after phase-1 pools are released (SBUF headroom).
        yTpool = ctx.enter_context(tc.tile_pool(name="yTpool", bufs=1))
        yT = [
            yTpool.tile([P, T], BF16, name=f"yT{j}", tag=f"yT{j}")
            for j in range(4)
        ]

        # ------------------------- Phase 2: attention ----------------------
        # qi rows are processed in groups of QG=8 blocks. Per (head, group):
        # for each row-block l: scores -> mask -> rowmax -> exp(+rowsum) ->
        # 1/sum -> normalize attn in place -> PE-transpose kj blocks into a
        # PSUM strip -> copy (split ACT||DVE) into aTBig, laid out
        # [l, chunk, kjblock, q]. Then one dense attn@v sweep with v
        # stationary accumulates y^T [64, QG*128] in PSUM via strided reads
        # of aTBig, and y^T lands in yT directly (no y transposes).
        QG = min(8, NT)
        NGRP = NT // QG
        NCH_MAX = (NT * P + 1023) // 1024
        L_STRIDE = NCH_MAX * 1024
        with tc.tile_pool(name="attnp", bufs=5) as attnp, \
             tc.tile_pool(name="aTbig", bufs=3) as aTbigp, \
             tc.tile_pool(name="stats", bufs=12) as stats, \
             tc.tile_pool(name="ps_s", bufs=2, space="PSUM") as ps_s, \
             tc.tile_pool(name="ps_tr", bufs=2, space="PSUM") as ps_tr, \
             tc.tile_pool(name="ps_y", bufs=1, space="PSUM") as ps_y:
            for hp in range(N_HEADS_LOC // 2):
                jt = hp
                qT_t = qkT[2 * jt]
                kT_t = qkT[2 * jt + 1]
                for g in range(NGRP):
                    aTBigs = {}
                    for h in (2 * hp, 2 * hp + 1):
                        aTBigs[h] = aTbigp.tile(
                            [P, QG * L_STRIDE], BF16, name="aTBig", tag="aTBig"
                        )
                    # interleave the two heads' row blocks: two independent
                    # softmax chains keep the PE fed while one chain sits in
                    # its DVE/ACT phase
                    for l in range(QG):
                      for h in (2 * hp, 2 * hp + 1):
                        po = (h % 2) * 64
                        aTBig = aTBigs[h]
                        qi = g * QG + l
                        nkj = qi + 1
                        cols_total = nkj * P
                        nch = (cols_total + 1023) // 1024
                        negmax = stats.tile([P, 1], F32, name="negmax", tag="negmax")
                        dsum = stats.tile([P, 2], F32, name="dsum", tag="dsum")
                        cmax = (
                            stats.tile([P, 2], F32, name="cmax", tag="cmax")
                            if nch > 1
                            else None
                        )
                        chunks = []
                        for chi in range(nch):
                            c0 = chi * 1024
                            ccols = min(1024, cols_total - c0)
                            ps = ps_s.tile([P, 1024], F32, name="ps", tag="s")
                            chunks.append((ps, c0, ccols))
                            for g0 in range(0, ccols, 512):
                                N = min(512, ccols - g0)
                                nc.tensor.matmul(
                                    ps[:, g0:g0 + N],
                                    qT_t[po:po + 64, qi * P:(qi + 1) * P],
                                    kT_t[po:po + 64, c0 + g0:c0 + g0 + N],
                                    start=True,
                                    stop=True,
                                )
                            if c0 + ccols == cols_total:
                                dg = ccols - P
                                nc.vector.tensor_add(
                                    ps[:, dg:dg + P], ps[:, dg:dg + P], causal
                                )
                            if nch > 1:
                                nc.vector.tensor_reduce(
                                    cmax[:, chi:chi + 1], ps[:, :ccols],
                                    axis=AX.X, op=OP.max,
                                )
                        if nch > 1:
                            nc.vector.tensor_reduce(
                                negmax, cmax[:, :nch], axis=AX.X, op=OP.max,
                                negate=True,
                            )
                        else:
                            ps0, _, cc0 = chunks[0]
                            nc.vector.tensor_reduce(
                                negmax, ps0[:, :cc0], axis=AX.X, op=OP.max,
                                negate=True,
                            )
                        at_tiles = []
                        for chi, (ps, c0, ccols) in enumerate(chunks):
                            at = attnp.tile([P, 1024], BF16, name="at", tag="at")
                            nc.scalar.activation(
                                at[:, :ccols], ps[:, :ccols], AF.Exp,
                                bias=negmax, scale=1.0,
                                accum_out=dsum[:, chi:chi + 1],
                            )
                            at_tiles.append(at)
                        rec = stats.tile([P, 1], F32, name="rec", tag="rec")
                        if nch > 1:
                            den = stats.tile([P, 1], F32, name="den", tag="den")
                            nc.vector.tensor_add(den, dsum[:, 0:1], dsum[:, 1:2])
                            nc.vector.reciprocal(rec, den)
                        else:
                            nc.vector.reciprocal(rec, dsum[:, 0:1])
                        # normalize attn in place, then transpose per chunk
                        for chi, (ps, c0, ccols) in enumerate(chunks):
                            at = at_tiles[chi]
                            nc.vector.tensor_scalar_mul(
                                at[:, :ccols], at[:, :ccols], rec
                            )
                            nb = ccols // P
                            ptr = ps_tr.tile([P, 1024], BF16, name="ptr", tag="tr")
                            for i in range(nb):
                                nc.tensor.transpose(
                                    ptr[:, i * P:(i + 1) * P],
                                    at[:, i * P:(i + 1) * P],
                                    ident_b,
                                )
                            base = l * L_STRIDE + chi * 1024
                            hw = min(512, ccols)
                            nc.scalar.copy(
                                aTBig[:, base:base + hw], ptr[:, :hw]
                            )
                            if ccols > 512:
                                nc.vector.tensor_copy(
                                    aTBig[:, base + 512:base + ccols],
                                    ptr[:, 512:ccols],
                                )
                    # dense attn @ v sweeps, v stationary, y^T accumulates
                    for h in (2 * hp, 2 * hp + 1):
                      po = (h % 2) * 64
                      aT3 = aTBigs[h].rearrange("p (l r) -> p l r", l=QG)
                      pyt = ps_y.tile([64, QG * P], F32, name="pyt", tag="yt")
                      nb_tot = (g + 1) * QG
                      for b in range(nb_tot):
                            chi, i = b // 8, b % 8
                            l0 = max(0, b - g * QG)
                            segs = []
                            for s0, s1 in ((0, 4), (4, QG)):
                                a, bnd = max(l0, s0), s1
                                if a < bnd:
                                    segs.append((a, bnd - a))
                            for ls, nl in segs:
                                rhs = aT3[:, ls:ls + nl,
                                          chi * 1024 + i * P: chi * 1024 + (i + 1) * P]
                                nc.tensor.matmul(
                                    pyt[:, ls * P:(ls + nl) * P],
                                    v_sb[b][:, h * DH:(h + 1) * DH],
                                    rhs,
                                    start=(b == 0),
                                    stop=(b == nb_tot - 1),
                                    skip_group_check=True,
                                )
                      copy_psum(
                          yT[jt][po:po + 64, g * QG * P:(g + 1) * QG * P], pyt
                      )

        # ---------------- Phase 3: output projection -----------------------
        with tc.tile_pool(name="p3", bufs=2) as p3, \
             tc.tile_pool(name="wpb", bufs=1) as wpb, \
             tc.tile_pool(name="ps_o", bufs=4, space="PSUM") as ps_o:
            wpb_t = []
            for j in range(4):
                wps = p3.tile([P, C_IN], F32, name="wps", tag="wps")
                nc.sync.dma_start(out=wps, in_=wp_d[j * P:(j + 1) * P, :])
                wb = wpb.tile([P, C_IN], BF16, name=f"wb{j}", tag=f"wb{j}")
                nc.vector.tensor_copy(wb, wps)
                wpb_t.append(wb)
            for t in range(NT):
                po_ = [
                    ps_o.tile([P, 512], F32, name="po", tag="po")
                    for _ in range(2)
                ]
                for j in range(4):
                    for cc in range(2):
                        nc.tensor.matmul(
                            po_[cc],
                            yT[j][:, t * P:(t + 1) * P],
                            wpb_t[j][:, cc * 512:(cc + 1) * 512],
                            start=(j == 0),
                            stop=(j == 3),
                        )
                ob = p3.tile([P, C_IN], F32, name="ob", tag="ob")
                copy_psum(ob[:, 0:512], po_[0])
                copy_psum(ob[:, 512:1024], po_[1])
                nc.sync.dma_start(out=out_d[t * P:(t + 1) * P, :], in_=ob)

    return nc


# ---------------------------------------------------------------------------
# Host entry point
# ---------------------------------------------------------------------------

_NC_CACHE: dict[int, bass.Bass] = {}


def _get_nc(T: int) -> bass.Bass:
    if T not in _NC_CACHE:
        _NC_CACHE[T] = build_attention_nc(T)
    return _NC_CACHE[T]


def make_in_maps(x, W_qkv, W_proj):
    x = np.asarray(x, dtype=np.float32)
    W_qkv = np.asarray(W_qkv, dtype=np.float32)
    W_proj = np.asarray(W_proj, dtype=np.float32)
    B = x.shape[0]
    in_maps = []
    for c in range(2 * B):
        b, hh = divmod(c, 2)
        h0 = hh * N_HEADS_LOC
        col = h0 * DH
        in_maps.append({
            "x": np.ascontiguousarray(x[b]),
            # exact power-of-two fold of the (inverse) softmax scale into Wq
            "wq": np.ascontiguousarray(W_qkv[:, col:col + DQ] * 8.0),
            "wk": np.ascontiguousarray(W_qkv[:, C_IN + col:C_IN + col + DQ]),
            "wv": np.ascontiguousarray(
                W_qkv[:, 2 * C_IN + col:2 * C_IN + col + DQ]
            ),
            "wp": np.ascontiguousarray(W_proj[col:col + DQ, :]),
        })
    return in_maps


def run_spmd(nc, in_maps, trace=False, **kwargs):
    return bass_utils.run_bass_kernel_spmd(
        nc, in_maps, core_ids=list(range(len(in_maps))), trace=trace, **kwargs
    )


def kernel(x, W_qkv, W_proj):
    x = np.asarray(x, dtype=np.float32)
    B, T, _ = x.shape
    nc = _get_nc(T)
    in_maps = make_in_maps(x, W_qkv, W_proj)
    res = run_spmd(nc, in_maps)
    out = np.empty((B, T, C_IN), dtype=np.float32)
    for b in range(B):
        out[b] = res.results[2 * b]["out"] + res.results[2 * b + 1]["out"]
    return out
